# revision 4
# baseline (speedup 1.0000x reference)
"""BitLinear (ternary 1.58-bit quantized linear) Trainium2 kernel, 8 cores.

y = x @ (sign(w) * (|w| > t))^T * scale + bias
  t     = k-th smallest |w| (k = n/2, exact order statistic)
  scale = mean(|w| over kept weights)

Strategy (data-parallel over batch rows):
 - every core holds the full weight, a disjoint 1/8 row-shard of x, and a
   disjoint 1/8 of the weight for the threshold histogramming.
 - threshold: exact value-space bisection on fp32 |w| (10 rounds x 3
   midpoints, counts AllReduce'd across cores). All ops stay fp32/integer:
   DVE data converters are lossy (bf16 mantissa) and must be avoided.
 - matmul: x is pre-scaled by `scale`, split into bf16 hi+lo (exact-ternary
   weights in bf16), accumulated in fp32 PSUM -> fp32-class accuracy at
   bf16 PE rate. Stationary = transposed ternary tiles, moving = x^T.
 - psum [o,b] tiles are scaled+biased on the scalar engine, transposed back
   to [b,o] via PE transpose, and DMA'd straight to the output slab.
"""
import numpy as np
import concourse.bass as bass
import concourse.mybir as mybir
import concourse.tile as tile
from concourse import bacc
from concourse.bass_utils import run_bass_kernel_spmd
from concourse.masks import make_identity

dt = mybir.dt
OP = mybir.AluOpType
AX = mybir.AxisListType.X
AF = mybir.ActivationFunctionType

NCORES = 8
P = 128
SPARSITY = 0.5
BIG = 1e9


def _f32_bits_off(val, off):
    return float(np.uint32(int(np.float32(val).view(np.uint32)) + off).view(np.float32))


def build(IF=4096, OF=4096, BLOC=1024, ncores=NCORES, rounds=10, win=131072):
    """Emit the SPMD program. Shapes: w [OF,IF], x-shard [BLOC,IF],
    whist [128, OF*IF/ncores/128], bias [OF,1] -> y [BLOC, OF]."""
    N = OF * IF
    K_RANK = int(N * SPARSITY)
    HF = N // ncores // P
    bound = 1.0 / np.sqrt(IF)
    wlo = _f32_bits_off(bound / 2, -win)
    whi = _f32_bits_off(bound / 2, +win)
    n_ot = OF // P          # output tiles
    n_ig = IF // 512        # weight column groups
    n_bt = BLOC // P        # x row tiles
    n_cc = IF // 1024       # x stage column chunks
    n_bh = BLOC // 512      # psum halves per o-tile
    assert BLOC % 512 == 0 and IF % 1024 == 0 and OF % P == 0

    nc = bacc.Bacc("TRN2", target_bir_lowering=False, debug=False,
                   num_devices=ncores)
    whist = nc.dram_tensor("whist", [P, HF], dt.float32, kind="ExternalInput").ap()
    w_in = nc.dram_tensor("w", [OF, IF], dt.float32, kind="ExternalInput").ap()
    x_in = nc.dram_tensor("x", [BLOC, IF], dt.float32, kind="ExternalInput").ap()
    b_in = nc.dram_tensor("bias", [OF, 1], dt.float32, kind="ExternalInput").ap()
    y_out = nc.dram_tensor("y", [BLOC, OF], dt.float32, kind="ExternalOutput").ap()

    with tile.TileContext(nc) as tc:
        with tc.tile_pool(name="bigp", bufs=2) as bigp, \
             tc.tile_pool(name="smallp", bufs=1) as smallp, \
             tc.tile_pool(name="xstage", bufs=2) as xsp, \
             tc.tile_pool(name="ternp", bufs=3) as ternp, \
             tc.tile_pool(name="outp", bufs=3) as outp, \
             tc.tile_pool(name="pmm", bufs=2 * n_bh, space="PSUM") as pmm, \
             tc.tile_pool(name="ptr", bufs=2, space="PSUM") as ptr, \
             tc.tile_pool(name="psel", bufs=2, space="PSUM") as psel, \
             tc.tile_pool(name="dramp", bufs=1, space="DRAM") as dramp:

            # ---------------- Phase A: threshold + scale ----------------
            a = bigp.tile([P, HF], dt.float32, tag="bigbuf")
            nc.sync.dma_start(out=a, in_=whist)
            # |w| in place (integer ALU, exact)
            nc.vector.tensor_scalar(out=a[:].bitcast(dt.int32),
                                    in0=a[:].bitcast(dt.int32),
                                    scalar1=0x7FFFFFFF, scalar2=None,
                                    op0=OP.bitwise_and)
            junk8 = smallp.tile([P, HF], dt.uint8, tag="junk")

            ones = smallp.tile([P, 1], dt.float32)
            nc.vector.memset(ones[:], 1.0)
            iota5 = smallp.tile([1, 5], dt.float32)
            for j in range(5):
                nc.vector.memset(iota5[:, j:j + 1], float(j))

            LH = smallp.tile([1, 2], dt.float32)
            nc.vector.memset(LH[:, 0:1], wlo)
            nc.vector.memset(LH[:, 1:2], whi)
            m_row = smallp.tile([1, 5], dt.float32)
            thr_rep = smallp.tile([P, 3], dt.float32)
            cnt128 = smallp.tile([P, 3], dt.float32)
            part = smallp.tile([3, 1], dt.float32)
            g_row = smallp.tile([1, 3], dt.float32)
            s_row = smallp.tile([1, 3], dt.float32)
            r11 = smallp.tile([1, 1], dt.float32)
            e_row = smallp.tile([1, 5], dt.float32)
            tmp5 = smallp.tile([1, 5], dt.float32)
            cle = smallp.tile([1, 1], dt.float32)
            gprev = smallp.tile([1, 1], dt.float32)
            zrow = smallp.tile([1, 3], dt.float32)

            bounce_in = dramp.tile([3, 1], dt.float32)
            bounce_out = dramp.tile([3, 1], dt.float32)
            rg = [list(range(ncores))]

            for rnd in range(rounds):
                nc.vector.tensor_copy(out=m_row[:, 0:1], in_=LH[:, 0:1])
                nc.vector.tensor_copy(out=m_row[:, 4:5], in_=LH[:, 1:2])
                nc.vector.tensor_tensor(out=m_row[:, 2:3], in0=LH[:, 0:1],
                                        in1=LH[:, 1:2], op=OP.add)
                nc.vector.tensor_scalar(out=m_row[:, 2:3], in0=m_row[:, 2:3],
                                        scalar1=0.5, scalar2=None, op0=OP.mult)
                nc.vector.tensor_tensor(out=m_row[:, 1:2], in0=m_row[:, 0:1],
                                        in1=m_row[:, 2:3], op=OP.add)
                nc.vector.tensor_scalar(out=m_row[:, 1:2], in0=m_row[:, 1:2],
                                        scalar1=0.5, scalar2=None, op0=OP.mult)
                nc.vector.tensor_tensor(out=m_row[:, 3:4], in0=m_row[:, 2:3],
                                        in1=m_row[:, 4:5], op=OP.add)
                nc.vector.tensor_scalar(out=m_row[:, 3:4], in0=m_row[:, 3:4],
                                        scalar1=0.5, scalar2=None, op0=OP.mult)
                nc.gpsimd.partition_broadcast(thr_rep[:], m_row[:, 1:4])
                for j in range(3):
                    nc.vector.tensor_scalar(
                        out=junk8[:], in0=a[:], scalar1=thr_rep[:, j:j + 1],
                        scalar2=0.0, op0=OP.is_le, op1=OP.add,
                        accum_out=cnt128[:, j:j + 1])
                ps = psel.tile([3, 1], dt.float32, tag="sel", space="PSUM")
                nc.tensor.matmul(out=ps[:], lhsT=cnt128[:], rhs=ones[:],
                                 start=True, stop=True)
                nc.vector.tensor_copy(out=part[:], in_=ps[:])
                nc.sync.dma_start(out=bounce_in[:], in_=part[:])
                nc.gpsimd.collective_compute(
                    "AllReduce", OP.add, replica_groups=rg,
                    ins=[bounce_in[:]], outs=[bounce_out[:]])
                nc.sync.dma_start(out=g_row[:],
                                  in_=bounce_out[:].rearrange("a b -> b a"))
                nc.vector.tensor_scalar(out=s_row[:], in0=g_row[:],
                                        scalar1=float(K_RANK), scalar2=None,
                                        op0=OP.is_lt)
                nc.vector.tensor_reduce(out=r11[:], in_=s_row[:], axis=AX,
                                        op=OP.add)
                nc.vector.tensor_scalar(out=e_row[:], in0=iota5[:],
                                        scalar1=r11[:, 0:1], scalar2=None,
                                        op0=OP.is_equal)
                nc.vector.tensor_tensor(out=tmp5[:], in0=m_row[:], in1=e_row[:],
                                        op=OP.mult)
                nc.vector.tensor_reduce(out=LH[:, 0:1], in_=tmp5[:], axis=AX,
                                        op=OP.add)
                nc.vector.tensor_scalar(out=e_row[:], in0=iota5[:],
                                        scalar1=r11[:, 0:1], scalar2=1.0,
                                        op0=OP.subtract, op1=OP.is_equal)
                nc.vector.tensor_tensor(out=tmp5[:], in0=m_row[:], in1=e_row[:],
                                        op=OP.mult)
                nc.vector.tensor_reduce(out=LH[:, 1:2], in_=tmp5[:], axis=AX,
                                        op=OP.add)
                nc.vector.tensor_scalar(out=zrow[:], in0=s_row[:], scalar1=BIG,
                                        scalar2=None, op0=OP.mult)
                nc.vector.tensor_tensor(out=zrow[:], in0=zrow[:], in1=g_row[:],
                                        op=OP.add)
                nc.vector.tensor_reduce(out=cle[:], in_=zrow[:], axis=AX,
                                        op=OP.min)
                if rnd == 0:
                    nc.vector.tensor_copy(out=gprev[:], in_=cle[:])
                else:
                    nc.vector.tensor_tensor(out=cle[:], in0=cle[:],
                                            in1=gprev[:], op=OP.min)
                    nc.vector.tensor_copy(out=gprev[:], in_=cle[:])

            t11 = smallp.tile([1, 1], dt.float32)
            nc.vector.tensor_copy(out=t11[:], in_=LH[:, 1:2])
            t_rep = smallp.tile([P, 1], dt.float32)
            nc.gpsimd.partition_broadcast(t_rep[:], t11[:])

            # S = sum(|w| where > t); in-place masked write (a is dead after)
            spart = smallp.tile([P, 1], dt.float32)
            nc.vector.scalar_tensor_tensor(
                out=a[:], in0=a[:], scalar=t_rep[:, :1], in1=a[:],
                op0=OP.is_gt, op1=OP.mult, accum_out=spart[:])
            ps2 = psel.tile([3, 1], dt.float32, tag="sel", space="PSUM")
            nc.tensor.matmul(out=ps2[:1, :], lhsT=spart[:], rhs=ones[:],
                             start=True, stop=True)
            sloc = smallp.tile([1, 1], dt.float32)
            nc.vector.tensor_copy(out=sloc[:], in_=ps2[:1, :])
            sb_in = dramp.tile([1, 1], dt.float32)
            sb_out = dramp.tile([1, 1], dt.float32)
            nc.sync.dma_start(out=sb_in[:], in_=sloc[:])
            nc.gpsimd.collective_compute(
                "AllReduce", OP.add, replica_groups=rg,
                ins=[sb_in[:]], outs=[sb_out[:]])
            sglob = smallp.tile([1, 1], dt.float32)
            nc.sync.dma_start(out=sglob[:], in_=sb_out[:])

            # scale = S / max(N - cnt_le, 1)
            denom = smallp.tile([1, 1], dt.float32)
            nc.vector.tensor_scalar(out=denom[:], in0=cle[:], scalar1=-1.0,
                                    scalar2=float(N), op0=OP.mult, op1=OP.add)
            nc.vector.tensor_scalar(out=denom[:], in0=denom[:], scalar1=1.0,
                                    scalar2=None, op0=OP.max)
            rden = smallp.tile([1, 1], dt.float32)
            nc.vector.reciprocal(out=rden[:], in_=denom[:])
            scl = smallp.tile([1, 1], dt.float32)
            nc.vector.tensor_tensor(out=scl[:], in0=sglob[:], in1=rden[:],
                                    op=OP.mult)
            scale_rep = smallp.tile([P, 1], dt.float32)
            nc.gpsimd.partition_broadcast(scale_rep[:], scl[:])

            # ---------------- Phase B: stage x^T (scaled, bf16 hi/lo) ----
            xhT = bigp.tile([P, IF // P, BLOC], dt.bfloat16, tag="bigbuf")
            xlT = bigp.tile([P, IF // P, BLOC], dt.bfloat16, tag="bigbuf")
            for bt in range(n_bt):
                for cc in range(n_cc):
                    xs = xsp.tile([P, 1024], dt.float32, tag="xs")
                    nc.sync.dma_start(
                        out=xs, in_=x_in[bt * P:(bt + 1) * P,
                                         cc * 1024:(cc + 1) * 1024])
                    xsc = xsp.tile([P, 1024], dt.float32, tag="xsc")
                    nc.vector.tensor_scalar(out=xsc[:], in0=xs[:],
                                            scalar1=scale_rep[:, :1],
                                            scalar2=None, op0=OP.mult)
                    xhb = xsp.tile([P, 1024], dt.bfloat16, tag="xhb")
                    nc.vector.tensor_copy(out=xhb[:], in_=xsc[:])
                    xlb = xsp.tile([P, 1024], dt.bfloat16, tag="xlb")
                    nc.vector.tensor_tensor(out=xlb[:], in0=xsc[:], in1=xhb[:],
                                            op=OP.subtract)
                    for q in range(8):
                        ic = cc * 8 + q
                        nc.sync.dma_start_transpose(
                            out=xhT[:, ic, bt * P:(bt + 1) * P],
                            in_=xhb[:, q * P:(q + 1) * P])
                        nc.sync.dma_start_transpose(
                            out=xlT[:, ic, bt * P:(bt + 1) * P],
                            in_=xlb[:, q * P:(q + 1) * P])

            # ---------------- Phase C: quantize + matmul + output --------
            ident = smallp.tile([P, P], dt.float32)
            make_identity(nc, ident[:])
            n_ic = IF // P
            for ot in range(n_ot):
                bias_t = outp.tile([P, 1], dt.float32, tag="bias")
                nc.sync.dma_start(out=bias_t,
                                  in_=b_in[ot * P:(ot + 1) * P, :])
                psb = [pmm.tile([P, 512], dt.float32, tag="mm", space="PSUM",
                                name=f"psb{ot}_{bh}")
                       for bh in range(n_bh)]
                for ig in range(n_ig):
                    wt = ternp.tile([P, 512], dt.float32, tag="wt")
                    nc.sync.dma_start(
                        out=wt, in_=w_in[ot * P:(ot + 1) * P,
                                         ig * 512:(ig + 1) * 512])
                    aa = ternp.tile([P, 512], dt.float32, tag="aa")
                    nc.vector.tensor_scalar(out=aa[:].bitcast(dt.int32),
                                            in0=wt[:].bitcast(dt.int32),
                                            scalar1=0x7FFFFFFF, scalar2=None,
                                            op0=OP.bitwise_and)
                    sg = ternp.tile([P, 512], dt.float32, tag="sg")
                    nc.vector.tensor_scalar(out=sg[:].bitcast(dt.int32),
                                            in0=wt[:].bitcast(dt.int32),
                                            scalar1=-0x80000000,
                                            scalar2=0x3F800000,
                                            op0=OP.bitwise_and,
                                            op1=OP.bitwise_or)
                    tb = ternp.tile([P, 512], dt.bfloat16, tag="tb")
                    nc.vector.scalar_tensor_tensor(
                        out=tb[:], in0=aa[:], scalar=t_rep[:, :1], in1=sg[:],
                        op0=OP.is_gt, op1=OP.mult)
                    for q in range(4):
                        ic = ig * 4 + q
                        tT = ternp.tile([P, P], dt.bfloat16, tag="tT")
                        nc.sync.dma_start_transpose(
                            out=tT[:], in_=tb[:, q * P:(q + 1) * P])
                        for bh in range(n_bh):
                            nc.tensor.matmul(
                                out=psb[bh][:], lhsT=tT[:],
                                rhs=xhT[:, ic, bh * 512:(bh + 1) * 512],
                                start=(ic == 0), stop=False)
                            nc.tensor.matmul(
                                out=psb[bh][:], lhsT=tT[:],
                                rhs=xlT[:, ic, bh * 512:(bh + 1) * 512],
                                start=False, stop=(ic == n_ic - 1))
                for bh in range(n_bh):
                    ysb = outp.tile([P, 512], dt.float32, tag="ysb")
                    nc.scalar.activation(ysb[:], psb[bh][:], AF.Identity,
                                         bias=bias_t[:, :1], scale=1.0)
                    for q in range(4):
                        pst = ptr.tile([P, P], dt.float32, tag="tr",
                                       space="PSUM")
                        nc.tensor.transpose(out=pst[:],
                                            in_=ysb[:, q * P:(q + 1) * P],
                                            identity=ident[:])
                        yo = outp.tile([P, P], dt.float32, tag="yo")
                        nc.vector.tensor_copy(out=yo[:], in_=pst[:])
                        row0 = bh * 512 + q * P
                        nc.sync.dma_start(
                            out=y_out[row0:row0 + P, ot * P:(ot + 1) * P],
                            in_=yo[:])
    nc.compile()
    return nc


_NC_CACHE = {}


def _get_nc():
    key = "full"
    if key not in _NC_CACHE:
        _NC_CACHE[key] = build()
    return _NC_CACHE[key]


def kernel(x, weight, bias):
    x = np.ascontiguousarray(np.asarray(x, dtype=np.float32))
    w = np.ascontiguousarray(np.asarray(weight, dtype=np.float32))
    b = np.ascontiguousarray(np.asarray(bias, dtype=np.float32))
    Bb, S, IF = x.shape
    OF = w.shape[0]
    xf = x.reshape(-1, IF)
    bloc = xf.shape[0] // NCORES
    rows = OF // NCORES
    nc = _get_nc()
    in_maps = []
    for c in range(NCORES):
        in_maps.append({
            "whist": np.ascontiguousarray(
                w[c * rows:(c + 1) * rows].reshape(P, -1)),
            "w": w,
            "x": np.ascontiguousarray(xf[c * bloc:(c + 1) * bloc]),
            "bias": b.reshape(-1, 1),
        })
    res = run_bass_kernel_spmd(nc, in_maps, core_ids=list(range(NCORES)))
    y = np.concatenate([res.results[c]["y"] for c in range(NCORES)], axis=0)
    return np.ascontiguousarray(y.reshape(Bb, S, OF))


# revision 15
# speedup vs baseline: 8032.1444x; 8032.1444x over previous
"""BitLinear (ternary 1.58-bit quantized linear) Trainium2 kernel, 8 cores.

y = x @ (sign(w) * (|w| > t))^T * scale + bias
  t     = k-th smallest |w| (k = n/2, exact order statistic)
  scale = mean(|w| over kept weights)

Strategy (data-parallel over batch rows):
 - every core holds the full weight, a disjoint 1/8 row-shard of x, and a
   disjoint 1/8 of the weight for the threshold histogramming.
 - threshold: exact value-space bisection on fp32 |w| (10 rounds x 3
   midpoints, counts AllReduce'd across cores). All ops stay fp32/integer:
   DVE data converters are lossy (bf16 mantissa) and must be avoided.
 - matmul: x is pre-scaled by `scale`, split into bf16 hi+lo (exact-ternary
   weights in bf16), accumulated in fp32 PSUM -> fp32-class accuracy at
   bf16 PE rate. Stationary = transposed ternary tiles, moving = x^T.
 - psum [o,b] tiles are scaled+biased on the scalar engine, transposed back
   to [b,o] via PE transpose, and DMA'd straight to the output slab.
"""
import numpy as np
import concourse.bass as bass
import concourse.mybir as mybir
import concourse.tile as tile
from concourse import bacc
from concourse.bass_utils import run_bass_kernel_spmd
from concourse.masks import make_identity

dt = mybir.dt
OP = mybir.AluOpType
AX = mybir.AxisListType.X
AF = mybir.ActivationFunctionType

NCORES = 8
P = 128
SPARSITY = 0.5
BIG = 1e9


def _f32_bits_off(val, off):
    return float(np.uint32(int(np.float32(val).view(np.uint32)) + off).view(np.float32))


def build(IF=4096, OF=4096, BLOC=1024, ncores=NCORES, rounds=10, win=131072,
          no_collective=False, hist_cores=None):
    """Emit the SPMD program. Shapes: w [OF,IF], x-shard [BLOC,IF],
    whist [128, OF*IF/ncores/128], bias [OF,1] -> y [BLOC, OF]."""
    N = OF * IF
    K_RANK = int(N * SPARSITY)
    HF = N // (hist_cores or ncores) // P
    bound = 1.0 / np.sqrt(IF)
    wlo = _f32_bits_off(bound / 2, -win)
    whi = _f32_bits_off(bound / 2, +win)
    n_ot = OF // P          # output tiles
    n_ig = IF // 512        # weight column groups
    n_bt = BLOC // P        # x row tiles
    n_cc = IF // 1024       # x stage column chunks
    n_bh = BLOC // 512      # psum halves per o-tile
    assert BLOC % 512 == 0 and IF % 1024 == 0 and OF % P == 0

    nc = bacc.Bacc("TRN2", target_bir_lowering=False, debug=False,
                   num_devices=ncores)
    whist = nc.dram_tensor("whist", [P, HF], dt.float32, kind="ExternalInput").ap()
    w_in = nc.dram_tensor("w", [OF, IF], dt.float32, kind="ExternalInput").ap()
    x_in = nc.dram_tensor("x", [BLOC, IF], dt.float32, kind="ExternalInput").ap()
    b_in = nc.dram_tensor("bias", [OF, 1], dt.float32, kind="ExternalInput").ap()
    y_out = nc.dram_tensor("y", [OF, BLOC], dt.float32, kind="ExternalOutput").ap()

    with tile.TileContext(nc) as tc:
        with tc.tile_pool(name="bigp", bufs=2) as bigp, \
             tc.tile_pool(name="smallp", bufs=1) as smallp, \
             tc.tile_pool(name="xstage", bufs=2) as xsp, \
             tc.tile_pool(name="ternp", bufs=3) as ternp, \
             tc.tile_pool(name="outp", bufs=3) as outp, \
             tc.tile_pool(name="pmm", bufs=8, space="PSUM") as pmm, \
             tc.tile_pool(name="dramp", bufs=1, space="DRAM") as dramp:

            # ---------------- Phase A: threshold + scale ----------------
            a = bigp.tile([P, HF], dt.float32, tag="bigbuf")
            nc.sync.dma_start(out=a, in_=whist)
            # |w| in place (integer ALU, exact)
            nc.vector.tensor_scalar(out=a[:].bitcast(dt.int32),
                                    in0=a[:].bitcast(dt.int32),
                                    scalar1=0x7FFFFFFF, scalar2=None,
                                    op0=OP.bitwise_and)
            junk8 = smallp.tile([P, HF], dt.uint8, tag="junk")

            iota5 = smallp.tile([1, 5], dt.float32)
            for j in range(5):
                nc.vector.memset(iota5[:, j:j + 1], float(j))

            LH = smallp.tile([1, 2], dt.float32)
            nc.vector.memset(LH[:, 0:1], wlo)
            nc.vector.memset(LH[:, 1:2], whi)
            m_row = smallp.tile([1, 5], dt.float32)
            thr_rep = smallp.tile([P, 3], dt.float32)
            cnt128 = smallp.tile([P, 3], dt.float32)
            cntA = smallp.tile([P, 3], dt.float32)
            g_row = smallp.tile([1, 3], dt.float32)
            s_row = smallp.tile([1, 3], dt.float32)
            r11 = smallp.tile([1, 1], dt.float32)
            e_row = smallp.tile([1, 5], dt.float32)
            tmp5 = smallp.tile([1, 5], dt.float32)
            cle = smallp.tile([1, 1], dt.float32)
            gprev = smallp.tile([1, 1], dt.float32)
            zrow = smallp.tile([1, 3], dt.float32)

            bounce_in = dramp.tile([1, 3], dt.float32)
            bounce_out = dramp.tile([1, 3], dt.float32)
            rg = [list(range(ncores))]

            for rnd in range(rounds):
                nc.vector.tensor_copy(out=m_row[:, 0:1], in_=LH[:, 0:1])
                nc.vector.tensor_copy(out=m_row[:, 4:5], in_=LH[:, 1:2])
                nc.vector.tensor_tensor(out=m_row[:, 2:3], in0=LH[:, 0:1],
                                        in1=LH[:, 1:2], op=OP.add)
                nc.vector.tensor_scalar(out=m_row[:, 2:3], in0=m_row[:, 2:3],
                                        scalar1=0.5, scalar2=None, op0=OP.mult)
                nc.vector.tensor_tensor(out=m_row[:, 1:2], in0=m_row[:, 0:1],
                                        in1=m_row[:, 2:3], op=OP.add)
                nc.vector.tensor_scalar(out=m_row[:, 1:2], in0=m_row[:, 1:2],
                                        scalar1=0.5, scalar2=None, op0=OP.mult)
                nc.vector.tensor_tensor(out=m_row[:, 3:4], in0=m_row[:, 2:3],
                                        in1=m_row[:, 4:5], op=OP.add)
                nc.vector.tensor_scalar(out=m_row[:, 3:4], in0=m_row[:, 3:4],
                                        scalar1=0.5, scalar2=None, op0=OP.mult)
                nc.gpsimd.partition_broadcast(thr_rep[:], m_row[:, 1:4])
                for j in range(3):
                    nc.vector.tensor_scalar(
                        out=junk8[:], in0=a[:], scalar1=thr_rep[:, j:j + 1],
                        scalar2=0.0, op0=OP.is_le, op1=OP.add,
                        accum_out=cnt128[:, j:j + 1])
                import concourse.bass_isa as bass_isa
                nc.gpsimd.partition_all_reduce(cntA[:], cnt128[:], channels=P,
                                               reduce_op=bass_isa.ReduceOp.add)
                nc.sync.dma_start(out=bounce_in[:], in_=cntA[:1, :])
                if no_collective:
                    nc.sync.dma_start(out=bounce_out[:], in_=bounce_in[:])
                else:
                    nc.gpsimd.collective_compute(
                        "AllReduce", OP.add, replica_groups=rg,
                        ins=[bounce_in[:]], outs=[bounce_out[:]])
                nc.sync.dma_start(out=g_row[:], in_=bounce_out[:])
                nc.vector.tensor_scalar(out=s_row[:], in0=g_row[:],
                                        scalar1=float(K_RANK), scalar2=None,
                                        op0=OP.is_lt)
                nc.vector.tensor_reduce(out=r11[:], in_=s_row[:], axis=AX,
                                        op=OP.add)
                nc.vector.tensor_scalar(out=e_row[:], in0=iota5[:],
                                        scalar1=r11[:, 0:1], scalar2=None,
                                        op0=OP.is_equal)
                nc.vector.tensor_tensor(out=tmp5[:], in0=m_row[:], in1=e_row[:],
                                        op=OP.mult)
                nc.vector.tensor_reduce(out=LH[:, 0:1], in_=tmp5[:], axis=AX,
                                        op=OP.add)
                nc.vector.tensor_scalar(out=e_row[:], in0=iota5[:],
                                        scalar1=r11[:, 0:1], scalar2=1.0,
                                        op0=OP.subtract, op1=OP.is_equal)
                nc.vector.tensor_tensor(out=tmp5[:], in0=m_row[:], in1=e_row[:],
                                        op=OP.mult)
                nc.vector.tensor_reduce(out=LH[:, 1:2], in_=tmp5[:], axis=AX,
                                        op=OP.add)
                nc.vector.tensor_scalar(out=zrow[:], in0=s_row[:], scalar1=BIG,
                                        scalar2=None, op0=OP.mult)
                nc.vector.tensor_tensor(out=zrow[:], in0=zrow[:], in1=g_row[:],
                                        op=OP.add)
                nc.vector.tensor_reduce(out=cle[:], in_=zrow[:], axis=AX,
                                        op=OP.min)
                if rnd == 0:
                    nc.vector.tensor_copy(out=gprev[:], in_=cle[:])
                else:
                    nc.vector.tensor_tensor(out=cle[:], in0=cle[:],
                                            in1=gprev[:], op=OP.min)
                    nc.vector.tensor_copy(out=gprev[:], in_=cle[:])

            t11 = smallp.tile([1, 1], dt.float32)
            nc.vector.tensor_copy(out=t11[:], in_=LH[:, 1:2])
            t_rep = smallp.tile([P, 1], dt.float32)
            nc.gpsimd.partition_broadcast(t_rep[:], t11[:])

            # S = sum(|w| where > t); in-place masked write (a is dead after)
            spart = smallp.tile([P, 1], dt.float32)
            nc.vector.scalar_tensor_tensor(
                out=a[:], in0=a[:], scalar=t_rep[:, :1], in1=a[:],
                op0=OP.is_gt, op1=OP.mult, accum_out=spart[:])
            spartA = smallp.tile([P, 1], dt.float32)
            import concourse.bass_isa as bass_isa
            nc.gpsimd.partition_all_reduce(spartA[:], spart[:], channels=P,
                                           reduce_op=bass_isa.ReduceOp.add)
            sloc = spartA
            sb_in = dramp.tile([1, 1], dt.float32)
            sb_out = dramp.tile([1, 1], dt.float32)
            nc.sync.dma_start(out=sb_in[:], in_=sloc[:1, :])
            if no_collective:
                nc.sync.dma_start(out=sb_out[:], in_=sb_in[:])
            else:
                nc.gpsimd.collective_compute(
                    "AllReduce", OP.add, replica_groups=rg,
                    ins=[sb_in[:]], outs=[sb_out[:]])
            sglob = smallp.tile([1, 1], dt.float32)
            nc.sync.dma_start(out=sglob[:], in_=sb_out[:])

            # scale = S / max(N - cnt_le, 1)
            denom = smallp.tile([1, 1], dt.float32)
            nc.vector.tensor_scalar(out=denom[:], in0=cle[:], scalar1=-1.0,
                                    scalar2=float(N), op0=OP.mult, op1=OP.add)
            nc.vector.tensor_scalar(out=denom[:], in0=denom[:], scalar1=1.0,
                                    scalar2=None, op0=OP.max)
            rden = smallp.tile([1, 1], dt.float32)
            nc.vector.reciprocal(out=rden[:], in_=denom[:])
            scl = smallp.tile([1, 1], dt.float32)
            nc.vector.tensor_tensor(out=scl[:], in0=sglob[:], in1=rden[:],
                                    op=OP.mult)
            scale_rep = smallp.tile([P, 1], dt.float32)
            nc.gpsimd.partition_broadcast(scale_rep[:], scl[:])

            # ---------------- Phase B: stage x^T (bf16 hi/lo via DRAM) ------
            # scale is applied at the output stage, so staging only needs x.
            xh_dram = dramp.tile([BLOC, IF], dt.bfloat16, name="xh_dram")
            xl_dram = dramp.tile([BLOC, IF], dt.bfloat16, name="xl_dram")
            CCX = min(1024, IF)
            CC = min(2048, IF)
            for bt in range(n_bt):
                for cc in range(IF // CCX):
                    xs = xsp.tile([P, CCX], dt.float32, tag="xs")
                    nc.sync.dma_start(
                        out=xs, in_=x_in[bt * P:(bt + 1) * P,
                                         cc * CCX:(cc + 1) * CCX])
                    xhb = xsp.tile([P, CCX], dt.bfloat16, tag="xhb")
                    nc.vector.tensor_copy(out=xhb[:], in_=xs[:])
                    xlb = xsp.tile([P, CCX], dt.bfloat16, tag="xlb")
                    nc.vector.tensor_tensor(out=xlb[:], in0=xs[:], in1=xhb[:],
                                            op=OP.subtract)
                    nc.sync.dma_start(
                        out=xh_dram[bt * P:(bt + 1) * P,
                                    cc * CCX:(cc + 1) * CCX],
                        in_=xhb[:])
                    nc.sync.dma_start(
                        out=xl_dram[bt * P:(bt + 1) * P,
                                    cc * CCX:(cc + 1) * CCX],
                        in_=xlb[:])
            xhT = bigp.tile([P, IF // P, BLOC], dt.bfloat16, tag="bigbuf")
            xlT = bigp.tile([P, IF // P, BLOC], dt.bfloat16, tag="bigbuf")
            for ic in range(IF // P):
                nc.sync.dma_start_transpose(
                    out=xhT[:, ic, :], in_=xh_dram[:, ic * P:(ic + 1) * P])
                nc.sync.dma_start_transpose(
                    out=xlT[:, ic, :], in_=xl_dram[:, ic * P:(ic + 1) * P])

            # ---------------- Phase C: quantize -> DRAM -> matmul ---------
            tern_dram = dramp.tile([OF, IF], dt.bfloat16, name="tern_dram")
            nt_rep = smallp.tile([P, 1], dt.float32)
            nc.vector.tensor_scalar(out=nt_rep[:], in0=t_rep[:], scalar1=-1.0,
                                    scalar2=None, op0=OP.mult)
            for wrow in range(OF // P):
                for cc in range(IF // CC):
                    wt = ternp.tile([P, CC], dt.float32, tag="wt", bufs=2)
                    nc.sync.dma_start(
                        out=wt, in_=w_in[wrow * P:(wrow + 1) * P,
                                         cc * CC:(cc + 1) * CC])
                    nb = ternp.tile([P, CC], dt.uint8, tag="nb", bufs=2)
                    nc.vector.tensor_scalar(out=nb[:], in0=wt[:],
                                            scalar1=nt_rep[:, :1],
                                            scalar2=None, op0=OP.is_lt)
                    tb = ternp.tile([P, CC], dt.bfloat16, tag="tb", bufs=2)
                    nc.vector.scalar_tensor_tensor(
                        out=tb[:], in0=wt[:], scalar=t_rep[:, :1], in1=nb[:],
                        op0=OP.is_gt, op1=OP.subtract)
                    nc.sync.dma_start(
                        out=tern_dram[wrow * P:(wrow + 1) * P,
                                      cc * CC:(cc + 1) * CC],
                        in_=tb[:])

            # bias for all o-tiles in one load: [128, n_ot]
            bias_all = smallp.tile([P, n_ot], dt.float32)
            nc.sync.dma_start(
                out=bias_all,
                in_=b_in.rearrange("(ot p) o -> p (ot o)", p=P))

            n_ic = IF // P
            OTG = 4
            for otg in range(n_ot // OTG):
                psb = [[pmm.tile([P, 512], dt.float32, tag="mm",
                                 space="PSUM", name=f"psb{otg}_{bh}_{g}")
                        for g in range(OTG)] for bh in range(n_bh)]
                for ic in range(n_ic):
                    ternT = ternp.tile([P, OTG * P], dt.bfloat16, tag="ternT",
                                       bufs=4)
                    nc.sync.dma_start_transpose(
                        out=ternT[:],
                        in_=tern_dram[otg * OTG * P:(otg + 1) * OTG * P,
                                      ic * P:(ic + 1) * P])
                    for bh in range(n_bh):
                        for g in range(OTG):
                            nc.tensor.matmul(
                                out=psb[bh][g][:],
                                lhsT=ternT[:, g * P:(g + 1) * P],
                                rhs=xhT[:, ic, bh * 512:(bh + 1) * 512],
                                start=(ic == 0), stop=False)
                            nc.tensor.matmul(
                                out=psb[bh][g][:],
                                lhsT=ternT[:, g * P:(g + 1) * P],
                                rhs=xlT[:, ic, bh * 512:(bh + 1) * 512],
                                start=False, stop=(ic == n_ic - 1))
                for bh in range(n_bh):
                    for g in range(OTG):
                        ot = otg * OTG + g
                        ysb = outp.tile([P, 512], dt.float32, tag="ysb")
                        nc.scalar.activation(ysb[:], psb[bh][g][:], AF.Identity,
                                             bias=bias_all[:, ot:ot + 1],
                                             scale=scale_rep[:, :1])
                        nc.sync.dma_start(
                            out=y_out[ot * P:(ot + 1) * P,
                                      bh * 512:(bh + 1) * 512],
                            in_=ysb[:])
    nc.compile()
    return nc


_NC_CACHE = {}


def _get_nc():
    key = "full"
    if key not in _NC_CACHE:
        _NC_CACHE[key] = build()
    return _NC_CACHE[key]


def kernel(x, weight, bias):
    x = np.ascontiguousarray(np.asarray(x, dtype=np.float32))
    w = np.ascontiguousarray(np.asarray(weight, dtype=np.float32))
    b = np.ascontiguousarray(np.asarray(bias, dtype=np.float32))
    Bb, S, IF = x.shape
    OF = w.shape[0]
    xf = x.reshape(-1, IF)
    bloc = xf.shape[0] // NCORES
    rows = OF // NCORES
    nc = _get_nc()
    in_maps = []
    for c in range(NCORES):
        in_maps.append({
            "whist": np.ascontiguousarray(
                w[c * rows:(c + 1) * rows].reshape(P, -1)),
            "w": w,
            "x": np.ascontiguousarray(xf[c * bloc:(c + 1) * bloc]),
            "bias": b.reshape(-1, 1),
        })
    res = run_bass_kernel_spmd(nc, in_maps, core_ids=list(range(NCORES)))
    yT = np.concatenate([res.results[c]["y"] for c in range(NCORES)], axis=1)
    return np.ascontiguousarray(yT.T).reshape(Bb, S, OF)


# revision 21
# speedup vs baseline: 8368.0684x; 1.0418x over previous
"""BitLinear (ternary 1.58-bit quantized linear) Trainium2 kernel, 8 cores.

y = x @ (sign(w) * (|w| > t))^T * scale + bias
  t     = k-th smallest |w| (k = n/2, exact order statistic)
  scale = mean(|w| over kept weights)

Strategy (data-parallel over batch rows):
 - every core holds the full weight, a disjoint 1/8 row-shard of x, and a
   disjoint 1/8 of the weight for the threshold histogramming.
 - threshold: exact value-space bisection on fp32 |w| (10 rounds x 3
   midpoints, counts AllReduce'd across cores). All ops stay fp32/integer:
   DVE data converters are lossy (bf16 mantissa) and must be avoided.
 - matmul: x is pre-scaled by `scale`, split into bf16 hi+lo (exact-ternary
   weights in bf16), accumulated in fp32 PSUM -> fp32-class accuracy at
   bf16 PE rate. Stationary = transposed ternary tiles, moving = x^T.
 - psum [o,b] tiles are scaled+biased on the scalar engine, transposed back
   to [b,o] via PE transpose, and DMA'd straight to the output slab.
"""
import numpy as np
import concourse.bass as bass
import concourse.mybir as mybir
import concourse.tile as tile
from concourse import bacc
from concourse.bass_utils import run_bass_kernel_spmd
from concourse.masks import make_identity

dt = mybir.dt
OP = mybir.AluOpType
AX = mybir.AxisListType.X
AF = mybir.ActivationFunctionType

NCORES = 8
P = 128
SPARSITY = 0.5
BIG = 1e9


def _f32_bits_off(val, off):
    return float(np.uint32(int(np.float32(val).view(np.uint32)) + off).view(np.float32))


def build(IF=4096, OF=4096, BLOC=1024, ncores=NCORES, rounds=10, win=131072,
          no_collective=False, hist_cores=None):
    """Emit the SPMD program. Shapes: w [OF,IF], x-shard [BLOC,IF],
    whist [128, OF*IF/ncores/128], bias [OF,1] -> y [BLOC, OF]."""
    N = OF * IF
    K_RANK = int(N * SPARSITY)
    HF = N // (hist_cores or ncores) // P
    bound = 1.0 / np.sqrt(IF)
    wlo = _f32_bits_off(bound / 2, -win)
    whi = _f32_bits_off(bound / 2, +win)
    n_ot = OF // P          # output tiles
    n_ig = IF // 512        # weight column groups
    n_bt = BLOC // P        # x row tiles
    n_cc = IF // 1024       # x stage column chunks
    n_bh = BLOC // 512      # psum halves per o-tile
    assert BLOC % 512 == 0 and IF % 1024 == 0 and OF % P == 0

    nc = bacc.Bacc("TRN2", target_bir_lowering=False, debug=False,
                   num_devices=ncores)
    whist = nc.dram_tensor("whist", [P, HF], dt.float32, kind="ExternalInput").ap()
    w_in = nc.dram_tensor("w", [OF, IF], dt.float32, kind="ExternalInput").ap()
    x_in = nc.dram_tensor("x", [BLOC, IF], dt.float32, kind="ExternalInput").ap()
    b_in = nc.dram_tensor("bias", [OF, 1], dt.float32, kind="ExternalInput").ap()
    y_out = nc.dram_tensor("y", [OF, BLOC], dt.float32, kind="ExternalOutput").ap()

    with tile.TileContext(nc) as tc:
        with tc.tile_pool(name="bigp", bufs=2) as bigp, \
             tc.tile_pool(name="smallp", bufs=1) as smallp, \
             tc.tile_pool(name="xstage", bufs=2) as xsp, \
             tc.tile_pool(name="ternp", bufs=3) as ternp, \
             tc.tile_pool(name="outp", bufs=3) as outp, \
             tc.tile_pool(name="pmm", bufs=8, space="PSUM") as pmm, \
             tc.tile_pool(name="dramp", bufs=1, space="DRAM") as dramp:

            # ---------------- Phase A: threshold + scale ----------------
            a = bigp.tile([P, HF], dt.float32, tag="bigbuf")
            nc.sync.dma_start(out=a, in_=whist)
            # |w| in place (integer ALU, exact)
            nc.vector.tensor_scalar(out=a[:].bitcast(dt.int32),
                                    in0=a[:].bitcast(dt.int32),
                                    scalar1=0x7FFFFFFF, scalar2=None,
                                    op0=OP.bitwise_and)
            junk8 = smallp.tile([P, HF], dt.uint8, tag="junk")

            iota5 = smallp.tile([1, 5], dt.float32)
            for j in range(5):
                nc.vector.memset(iota5[:, j:j + 1], float(j))

            LH = smallp.tile([1, 2], dt.float32)
            nc.vector.memset(LH[:, 0:1], wlo)
            nc.vector.memset(LH[:, 1:2], whi)
            m_row = smallp.tile([1, 5], dt.float32)
            thr_rep = smallp.tile([P, 3], dt.float32)
            cnt128 = smallp.tile([P, 3], dt.float32)
            cntA = smallp.tile([P, 3], dt.float32)
            g_row = smallp.tile([1, 3], dt.float32)
            s_row = smallp.tile([1, 3], dt.float32)
            r11 = smallp.tile([1, 1], dt.float32)
            e_row = smallp.tile([1, 5], dt.float32)
            tmp5 = smallp.tile([1, 5], dt.float32)
            cle = smallp.tile([1, 1], dt.float32)
            gprev = smallp.tile([1, 1], dt.float32)
            zrow = smallp.tile([1, 3], dt.float32)

            bounce_in = dramp.tile([1, 3], dt.float32)
            bounce_out = dramp.tile([1, 3], dt.float32)
            rg = [list(range(ncores))]

            if rounds == 0:
                nc.vector.memset(cle[:], float(K_RANK))
            for rnd in range(rounds):
                nc.vector.tensor_copy(out=m_row[:, 0:1], in_=LH[:, 0:1])
                nc.vector.tensor_copy(out=m_row[:, 4:5], in_=LH[:, 1:2])
                nc.vector.tensor_tensor(out=m_row[:, 2:3], in0=LH[:, 0:1],
                                        in1=LH[:, 1:2], op=OP.add)
                nc.vector.tensor_scalar(out=m_row[:, 2:3], in0=m_row[:, 2:3],
                                        scalar1=0.5, scalar2=None, op0=OP.mult)
                nc.vector.tensor_tensor(out=m_row[:, 1:2], in0=m_row[:, 0:1],
                                        in1=m_row[:, 2:3], op=OP.add)
                nc.vector.tensor_scalar(out=m_row[:, 1:2], in0=m_row[:, 1:2],
                                        scalar1=0.5, scalar2=None, op0=OP.mult)
                nc.vector.tensor_tensor(out=m_row[:, 3:4], in0=m_row[:, 2:3],
                                        in1=m_row[:, 4:5], op=OP.add)
                nc.vector.tensor_scalar(out=m_row[:, 3:4], in0=m_row[:, 3:4],
                                        scalar1=0.5, scalar2=None, op0=OP.mult)
                nc.gpsimd.partition_broadcast(thr_rep[:], m_row[:, 1:4])
                for j in range(3):
                    nc.vector.tensor_scalar(
                        out=junk8[:], in0=a[:], scalar1=thr_rep[:, j:j + 1],
                        scalar2=0.0, op0=OP.is_le, op1=OP.add,
                        accum_out=cnt128[:, j:j + 1])
                import concourse.bass_isa as bass_isa
                nc.gpsimd.partition_all_reduce(cntA[:], cnt128[:], channels=P,
                                               reduce_op=bass_isa.ReduceOp.add)
                nc.sync.dma_start(out=bounce_in[:], in_=cntA[:1, :3])
                if no_collective:
                    nc.sync.dma_start(out=bounce_out[:], in_=bounce_in[:])
                else:
                    nc.gpsimd.collective_compute(
                        "AllReduce", OP.add, replica_groups=rg,
                        ins=[bounce_in[:]], outs=[bounce_out[:]])
                nc.sync.dma_start(out=g_row[:], in_=bounce_out[:])
                nc.vector.tensor_scalar(out=s_row[:], in0=g_row[:],
                                        scalar1=float(K_RANK), scalar2=None,
                                        op0=OP.is_lt)
                nc.vector.tensor_reduce(out=r11[:], in_=s_row[:], axis=AX,
                                        op=OP.add)
                nc.vector.tensor_scalar(out=e_row[:], in0=iota5[:],
                                        scalar1=r11[:, 0:1], scalar2=None,
                                        op0=OP.is_equal)
                nc.vector.tensor_tensor(out=tmp5[:], in0=m_row[:], in1=e_row[:],
                                        op=OP.mult)
                nc.vector.tensor_reduce(out=LH[:, 0:1], in_=tmp5[:], axis=AX,
                                        op=OP.add)
                nc.vector.tensor_scalar(out=e_row[:], in0=iota5[:],
                                        scalar1=r11[:, 0:1], scalar2=1.0,
                                        op0=OP.subtract, op1=OP.is_equal)
                nc.vector.tensor_tensor(out=tmp5[:], in0=m_row[:], in1=e_row[:],
                                        op=OP.mult)
                nc.vector.tensor_reduce(out=LH[:, 1:2], in_=tmp5[:], axis=AX,
                                        op=OP.add)
                nc.vector.tensor_scalar(out=zrow[:], in0=s_row[:], scalar1=BIG,
                                        scalar2=None, op0=OP.mult)
                nc.vector.tensor_tensor(out=zrow[:], in0=zrow[:], in1=g_row[:],
                                        op=OP.add)
                nc.vector.tensor_reduce(out=cle[:], in_=zrow[:], axis=AX,
                                        op=OP.min)
                if rnd == 0:
                    nc.vector.tensor_copy(out=gprev[:], in_=cle[:])
                else:
                    nc.vector.tensor_tensor(out=cle[:], in0=cle[:],
                                            in1=gprev[:], op=OP.min)
                    nc.vector.tensor_copy(out=gprev[:], in_=cle[:])

            t11 = smallp.tile([1, 1], dt.float32)
            nc.vector.tensor_copy(out=t11[:], in_=LH[:, 1:2])
            t_rep = smallp.tile([P, 1], dt.float32)
            nc.gpsimd.partition_broadcast(t_rep[:], t11[:])

            # S = sum(|w| where > t); in-place masked write (a is dead after)
            spart = smallp.tile([P, 1], dt.float32)
            nc.vector.scalar_tensor_tensor(
                out=a[:], in0=a[:], scalar=t_rep[:, :1], in1=a[:],
                op0=OP.is_gt, op1=OP.mult, accum_out=spart[:])
            spartA = smallp.tile([P, 1], dt.float32)
            import concourse.bass_isa as bass_isa
            nc.gpsimd.partition_all_reduce(spartA[:], spart[:], channels=P,
                                           reduce_op=bass_isa.ReduceOp.add)
            sloc = spartA
            sb_in = dramp.tile([1, 1], dt.float32)
            sb_out = dramp.tile([1, 1], dt.float32)
            nc.sync.dma_start(out=sb_in[:], in_=sloc[:1, :])
            if no_collective:
                nc.sync.dma_start(out=sb_out[:], in_=sb_in[:])
            else:
                nc.gpsimd.collective_compute(
                    "AllReduce", OP.add, replica_groups=rg,
                    ins=[sb_in[:]], outs=[sb_out[:]])
            sglob = smallp.tile([1, 1], dt.float32)
            nc.sync.dma_start(out=sglob[:], in_=sb_out[:])

            # scale = S / max(N - cnt_le, 1)
            denom = smallp.tile([1, 1], dt.float32)
            nc.vector.tensor_scalar(out=denom[:], in0=cle[:], scalar1=-1.0,
                                    scalar2=float(N), op0=OP.mult, op1=OP.add)
            nc.vector.tensor_scalar(out=denom[:], in0=denom[:], scalar1=1.0,
                                    scalar2=None, op0=OP.max)
            rden = smallp.tile([1, 1], dt.float32)
            nc.vector.reciprocal(out=rden[:], in_=denom[:])
            scl = smallp.tile([1, 1], dt.float32)
            nc.vector.tensor_tensor(out=scl[:], in0=sglob[:], in1=rden[:],
                                    op=OP.mult)
            scale_rep = smallp.tile([P, 1], dt.float32)
            nc.gpsimd.partition_broadcast(scale_rep[:], scl[:])

            # ---------------- Phase B: stage x^T (bf16 hi/lo via DRAM) ------
            # scale is applied at the output stage, so staging only needs x.
            xh_dram = dramp.tile([BLOC, IF], dt.bfloat16, name="xh_dram")
            xl_dram = dramp.tile([BLOC, IF], dt.bfloat16, name="xl_dram")
            CCX = min(1024, IF)
            CC = min(2048, IF)
            for bt in range(n_bt):
                for cc in range(IF // CCX):
                    xs = xsp.tile([P, CCX], dt.float32, tag="xs")
                    nc.sync.dma_start(
                        out=xs, in_=x_in[bt * P:(bt + 1) * P,
                                         cc * CCX:(cc + 1) * CCX])
                    xhb = xsp.tile([P, CCX], dt.bfloat16, tag="xhb")
                    nc.scalar.copy(out=xhb[:], in_=xs[:])
                    xlb = xsp.tile([P, CCX], dt.bfloat16, tag="xlb")
                    nc.vector.tensor_tensor(out=xlb[:], in0=xs[:], in1=xhb[:],
                                            op=OP.subtract)
                    nc.sync.dma_start(
                        out=xh_dram[bt * P:(bt + 1) * P,
                                    cc * CCX:(cc + 1) * CCX],
                        in_=xhb[:])
                    nc.sync.dma_start(
                        out=xl_dram[bt * P:(bt + 1) * P,
                                    cc * CCX:(cc + 1) * CCX],
                        in_=xlb[:])
            xhT = bigp.tile([P, IF // P, BLOC], dt.bfloat16, tag="bigbuf")
            xlT = bigp.tile([P, IF // P, BLOC], dt.bfloat16, tag="bigbuf")
            for ic in range(IF // P):
                nc.sync.dma_start_transpose(
                    out=xhT[:, ic, :], in_=xh_dram[:, ic * P:(ic + 1) * P])
                nc.sync.dma_start_transpose(
                    out=xlT[:, ic, :], in_=xl_dram[:, ic * P:(ic + 1) * P])

            # ---------------- Phase C: quantize -> DRAM -> matmul ---------
            tern_dram = dramp.tile([OF, IF], dt.bfloat16, name="tern_dram")
            nt_rep = smallp.tile([P, 1], dt.float32)
            nc.vector.tensor_scalar(out=nt_rep[:], in0=t_rep[:], scalar1=-1.0,
                                    scalar2=None, op0=OP.mult)
            for wrow in range(OF // P):
                for cc in range(IF // CC):
                    wt = ternp.tile([P, CC], dt.float32, tag="wt", bufs=2)
                    nc.sync.dma_start(
                        out=wt, in_=w_in[wrow * P:(wrow + 1) * P,
                                         cc * CC:(cc + 1) * CC])
                    nb = ternp.tile([P, CC], dt.uint8, tag="nb", bufs=2)
                    nc.vector.tensor_scalar(out=nb[:], in0=wt[:],
                                            scalar1=nt_rep[:, :1],
                                            scalar2=None, op0=OP.is_lt)
                    tb = ternp.tile([P, CC], dt.bfloat16, tag="tb", bufs=2)
                    nc.vector.scalar_tensor_tensor(
                        out=tb[:], in0=wt[:], scalar=t_rep[:, :1], in1=nb[:],
                        op0=OP.is_gt, op1=OP.subtract)
                    nc.sync.dma_start(
                        out=tern_dram[wrow * P:(wrow + 1) * P,
                                      cc * CC:(cc + 1) * CC],
                        in_=tb[:])

            # bias for all o-tiles in one load: [128, n_ot]
            bias_all = smallp.tile([P, n_ot], dt.float32)
            nc.sync.dma_start(
                out=bias_all,
                in_=b_in.rearrange("(ot p) o -> p (ot o)", p=P))

            n_ic = IF // P
            OTG = 4
            for otg in range(n_ot // OTG):
                psb = [[pmm.tile([P, 512], dt.float32, tag="mm",
                                 space="PSUM", name=f"psb{otg}_{bh}_{g}")
                        for g in range(OTG)] for bh in range(n_bh)]
                for ic in range(n_ic):
                    ternT = ternp.tile([P, OTG * P], dt.bfloat16, tag="ternT",
                                       bufs=8)
                    nc.sync.dma_start_transpose(
                        out=ternT[:],
                        in_=tern_dram[otg * OTG * P:(otg + 1) * OTG * P,
                                      ic * P:(ic + 1) * P])
                    for bh in range(n_bh):
                        for g in range(OTG):
                            nc.tensor.matmul(
                                out=psb[bh][g][:],
                                lhsT=ternT[:, g * P:(g + 1) * P],
                                rhs=xhT[:, ic, bh * 512:(bh + 1) * 512],
                                start=(ic == 0), stop=False)
                            nc.tensor.matmul(
                                out=psb[bh][g][:],
                                lhsT=ternT[:, g * P:(g + 1) * P],
                                rhs=xlT[:, ic, bh * 512:(bh + 1) * 512],
                                start=False, stop=(ic == n_ic - 1))
                for g in range(OTG):
                    ot = otg * OTG + g
                    ysb = outp.tile([P, n_bh * 512], dt.float32, tag="ysb", bufs=2)
                    for bh in range(n_bh):
                        dst = ysb[:, bh * 512:(bh + 1) * 512]
                        if (g + bh) % 2 == 0:
                            nc.scalar.activation(dst, psb[bh][g][:],
                                                 AF.Identity,
                                                 bias=bias_all[:, ot:ot + 1],
                                                 scale=scale_rep[:, :1])
                        else:
                            nc.vector.tensor_scalar(
                                out=dst, in0=psb[bh][g][:],
                                scalar1=scale_rep[:, :1],
                                scalar2=bias_all[:, ot:ot + 1],
                                op0=OP.mult, op1=OP.add)
                    nc.sync.dma_start(
                        out=y_out[ot * P:(ot + 1) * P, :],
                        in_=ysb[:])
    nc.compile()
    return nc


_NC_CACHE = {}


def _get_nc():
    key = "full"
    if key not in _NC_CACHE:
        _NC_CACHE[key] = build()
    return _NC_CACHE[key]


def kernel(x, weight, bias):
    x = np.ascontiguousarray(np.asarray(x, dtype=np.float32))
    w = np.ascontiguousarray(np.asarray(weight, dtype=np.float32))
    b = np.ascontiguousarray(np.asarray(bias, dtype=np.float32))
    Bb, S, IF = x.shape
    OF = w.shape[0]
    xf = x.reshape(-1, IF)
    bloc = xf.shape[0] // NCORES
    rows = OF // NCORES
    nc = _get_nc()
    in_maps = []
    for c in range(NCORES):
        in_maps.append({
            "whist": np.ascontiguousarray(
                w[c * rows:(c + 1) * rows].reshape(P, -1)),
            "w": w,
            "x": np.ascontiguousarray(xf[c * bloc:(c + 1) * bloc]),
            "bias": b.reshape(-1, 1),
        })
    res = run_bass_kernel_spmd(nc, in_maps, core_ids=list(range(NCORES)))
    yT = np.concatenate([res.results[c]["y"] for c in range(NCORES)], axis=1)
    return np.ascontiguousarray(yT.T).reshape(Bb, S, OF)


# revision 23
# speedup vs baseline: 8597.0335x; 1.0274x over previous
"""BitLinear (ternary 1.58-bit quantized linear) Trainium2 kernel, 8 cores.

y = x @ (sign(w) * (|w| > t))^T * scale + bias
  t     = k-th smallest |w| (k = n/2, exact order statistic)
  scale = mean(|w| over kept weights)

Strategy (data-parallel over batch rows):
 - every core holds the full weight, a disjoint 1/8 row-shard of x, and a
   disjoint 1/8 of the weight for the threshold histogramming.
 - threshold: exact value-space bisection on fp32 |w| (10 rounds x 3
   midpoints, counts AllReduce'd across cores). All ops stay fp32/integer:
   DVE data converters are lossy (bf16 mantissa) and must be avoided.
 - matmul: x is pre-scaled by `scale`, split into bf16 hi+lo (exact-ternary
   weights in bf16), accumulated in fp32 PSUM -> fp32-class accuracy at
   bf16 PE rate. Stationary = transposed ternary tiles, moving = x^T.
 - psum [o,b] tiles are scaled+biased on the scalar engine, transposed back
   to [b,o] via PE transpose, and DMA'd straight to the output slab.
"""
import numpy as np
import concourse.bass as bass
import concourse.mybir as mybir
import concourse.tile as tile
from concourse import bacc
from concourse.bass_utils import run_bass_kernel_spmd
from concourse.masks import make_identity

dt = mybir.dt
OP = mybir.AluOpType
AX = mybir.AxisListType.X
AF = mybir.ActivationFunctionType

NCORES = 8
P = 128
SPARSITY = 0.5
BIG = 1e9


def _f32_bits_off(val, off):
    return float(np.uint32(int(np.float32(val).view(np.uint32)) + off).view(np.float32))


def build(IF=4096, OF=4096, BLOC=1024, ncores=NCORES, rounds=9, win=32768,
          no_collective=False, hist_cores=None):
    """Emit the SPMD program. Shapes: w [OF,IF], x-shard [BLOC,IF],
    whist [128, OF*IF/ncores/128], bias [OF,1] -> y [BLOC, OF]."""
    N = OF * IF
    K_RANK = int(N * SPARSITY)
    HF = N // (hist_cores or ncores) // P
    bound = 1.0 / np.sqrt(IF)
    wlo = _f32_bits_off(bound / 2, -win)
    whi = _f32_bits_off(bound / 2, +win)
    n_ot = OF // P          # output tiles
    n_ig = IF // 512        # weight column groups
    n_bt = BLOC // P        # x row tiles
    n_cc = IF // 1024       # x stage column chunks
    n_bh = BLOC // 512      # psum halves per o-tile
    assert BLOC % 512 == 0 and IF % 1024 == 0 and OF % P == 0

    nc = bacc.Bacc("TRN2", target_bir_lowering=False, debug=False,
                   num_devices=ncores)
    whist = nc.dram_tensor("whist", [P, HF], dt.float32, kind="ExternalInput").ap()
    w_in = nc.dram_tensor("w", [OF, IF], dt.float32, kind="ExternalInput").ap()
    x_in = nc.dram_tensor("x", [BLOC, IF], dt.float32, kind="ExternalInput").ap()
    b_in = nc.dram_tensor("bias", [OF, 1], dt.float32, kind="ExternalInput").ap()
    y_out = nc.dram_tensor("y", [OF, BLOC], dt.float32, kind="ExternalOutput").ap()

    with tile.TileContext(nc) as tc:
        with tc.tile_pool(name="bigp", bufs=2) as bigp, \
             tc.tile_pool(name="smallp", bufs=1) as smallp, \
             tc.tile_pool(name="xstage", bufs=2) as xsp, \
             tc.tile_pool(name="ternp", bufs=3) as ternp, \
             tc.tile_pool(name="outp", bufs=3) as outp, \
             tc.tile_pool(name="pmm", bufs=8, space="PSUM") as pmm, \
             tc.tile_pool(name="dramp", bufs=1, space="DRAM") as dramp:

            # ---------------- Phase A: threshold + scale ----------------
            a = bigp.tile([P, HF], dt.float32, tag="bigbuf")
            nc.sync.dma_start(out=a, in_=whist)
            # |w| in place (integer ALU, exact)
            nc.vector.tensor_scalar(out=a[:].bitcast(dt.int32),
                                    in0=a[:].bitcast(dt.int32),
                                    scalar1=0x7FFFFFFF, scalar2=None,
                                    op0=OP.bitwise_and)
            junk8 = smallp.tile([P, HF], dt.uint8, tag="junk")

            iota5 = smallp.tile([1, 5], dt.float32)
            for j in range(5):
                nc.vector.memset(iota5[:, j:j + 1], float(j))

            LH = smallp.tile([1, 2], dt.float32)
            nc.vector.memset(LH[:, 0:1], wlo)
            nc.vector.memset(LH[:, 1:2], whi)
            m_row = smallp.tile([1, 5], dt.float32)
            thr_rep = smallp.tile([P, 3], dt.float32)
            cnt128 = smallp.tile([P, 3], dt.float32)
            cntA = smallp.tile([P, 3], dt.float32)
            g_row = smallp.tile([1, 3], dt.float32)
            s_row = smallp.tile([1, 3], dt.float32)
            r11 = smallp.tile([1, 1], dt.float32)
            e_row = smallp.tile([1, 5], dt.float32)
            tmp5 = smallp.tile([1, 5], dt.float32)
            cle = smallp.tile([1, 1], dt.float32)
            gprev = smallp.tile([1, 1], dt.float32)
            zrow = smallp.tile([1, 3], dt.float32)

            bounce_in = dramp.tile([1, 3], dt.float32)
            bounce_out = dramp.tile([1, 3], dt.float32)
            rg = [list(range(ncores))]

            if rounds == 0:
                nc.vector.memset(cle[:], float(K_RANK))
            for rnd in range(rounds):
                nc.vector.tensor_copy(out=m_row[:, 0:1], in_=LH[:, 0:1])
                nc.vector.tensor_copy(out=m_row[:, 4:5], in_=LH[:, 1:2])
                nc.vector.tensor_tensor(out=m_row[:, 2:3], in0=LH[:, 0:1],
                                        in1=LH[:, 1:2], op=OP.add)
                nc.vector.tensor_scalar(out=m_row[:, 2:3], in0=m_row[:, 2:3],
                                        scalar1=0.5, scalar2=None, op0=OP.mult)
                nc.vector.tensor_tensor(out=m_row[:, 1:2], in0=m_row[:, 0:1],
                                        in1=m_row[:, 2:3], op=OP.add)
                nc.vector.tensor_scalar(out=m_row[:, 1:2], in0=m_row[:, 1:2],
                                        scalar1=0.5, scalar2=None, op0=OP.mult)
                nc.vector.tensor_tensor(out=m_row[:, 3:4], in0=m_row[:, 2:3],
                                        in1=m_row[:, 4:5], op=OP.add)
                nc.vector.tensor_scalar(out=m_row[:, 3:4], in0=m_row[:, 3:4],
                                        scalar1=0.5, scalar2=None, op0=OP.mult)
                nc.gpsimd.partition_broadcast(thr_rep[:], m_row[:, 1:4])
                for j in range(3):
                    nc.vector.tensor_scalar(
                        out=junk8[:], in0=a[:], scalar1=thr_rep[:, j:j + 1],
                        scalar2=0.0, op0=OP.is_le, op1=OP.add,
                        accum_out=cnt128[:, j:j + 1])
                import concourse.bass_isa as bass_isa
                nc.gpsimd.partition_all_reduce(cntA[:], cnt128[:], channels=P,
                                               reduce_op=bass_isa.ReduceOp.add)
                nc.sync.dma_start(out=bounce_in[:], in_=cntA[:1, :3])
                if no_collective:
                    nc.sync.dma_start(out=bounce_out[:], in_=bounce_in[:])
                else:
                    nc.gpsimd.collective_compute(
                        "AllReduce", OP.add, replica_groups=rg,
                        ins=[bounce_in[:]], outs=[bounce_out[:]])
                nc.sync.dma_start(out=g_row[:], in_=bounce_out[:])
                nc.vector.tensor_scalar(out=s_row[:], in0=g_row[:],
                                        scalar1=float(K_RANK), scalar2=None,
                                        op0=OP.is_lt)
                nc.vector.tensor_reduce(out=r11[:], in_=s_row[:], axis=AX,
                                        op=OP.add)
                nc.vector.tensor_scalar(out=e_row[:], in0=iota5[:],
                                        scalar1=r11[:, 0:1], scalar2=None,
                                        op0=OP.is_equal)
                nc.vector.tensor_tensor(out=tmp5[:], in0=m_row[:], in1=e_row[:],
                                        op=OP.mult)
                nc.vector.tensor_reduce(out=LH[:, 0:1], in_=tmp5[:], axis=AX,
                                        op=OP.add)
                nc.vector.tensor_scalar(out=e_row[:], in0=iota5[:],
                                        scalar1=r11[:, 0:1], scalar2=1.0,
                                        op0=OP.subtract, op1=OP.is_equal)
                nc.vector.tensor_tensor(out=tmp5[:], in0=m_row[:], in1=e_row[:],
                                        op=OP.mult)
                nc.vector.tensor_reduce(out=LH[:, 1:2], in_=tmp5[:], axis=AX,
                                        op=OP.add)
                nc.vector.tensor_scalar(out=zrow[:], in0=s_row[:], scalar1=BIG,
                                        scalar2=None, op0=OP.mult)
                nc.vector.tensor_tensor(out=zrow[:], in0=zrow[:], in1=g_row[:],
                                        op=OP.add)
                nc.vector.tensor_reduce(out=cle[:], in_=zrow[:], axis=AX,
                                        op=OP.min)
                if rnd == 0:
                    nc.vector.tensor_copy(out=gprev[:], in_=cle[:])
                else:
                    nc.vector.tensor_tensor(out=cle[:], in0=cle[:],
                                            in1=gprev[:], op=OP.min)
                    nc.vector.tensor_copy(out=gprev[:], in_=cle[:])

            t11 = smallp.tile([1, 1], dt.float32)
            nc.vector.tensor_copy(out=t11[:], in_=LH[:, 1:2])
            t_rep = smallp.tile([P, 1], dt.float32)
            nc.gpsimd.partition_broadcast(t_rep[:], t11[:])

            # S = sum(|w| where > t); in-place masked write (a is dead after)
            spart = smallp.tile([P, 1], dt.float32)
            nc.vector.scalar_tensor_tensor(
                out=a[:], in0=a[:], scalar=t_rep[:, :1], in1=a[:],
                op0=OP.is_gt, op1=OP.mult, accum_out=spart[:])
            spartA = smallp.tile([P, 1], dt.float32)
            import concourse.bass_isa as bass_isa
            nc.gpsimd.partition_all_reduce(spartA[:], spart[:], channels=P,
                                           reduce_op=bass_isa.ReduceOp.add)
            sloc = spartA
            sb_in = dramp.tile([1, 1], dt.float32)
            sb_out = dramp.tile([1, 1], dt.float32)
            nc.sync.dma_start(out=sb_in[:], in_=sloc[:1, :])
            if no_collective:
                nc.sync.dma_start(out=sb_out[:], in_=sb_in[:])
            else:
                nc.gpsimd.collective_compute(
                    "AllReduce", OP.add, replica_groups=rg,
                    ins=[sb_in[:]], outs=[sb_out[:]])
            sglob = smallp.tile([1, 1], dt.float32)
            nc.sync.dma_start(out=sglob[:], in_=sb_out[:])

            # scale = S / max(N - cnt_le, 1)
            denom = smallp.tile([1, 1], dt.float32)
            nc.vector.tensor_scalar(out=denom[:], in0=cle[:], scalar1=-1.0,
                                    scalar2=float(N), op0=OP.mult, op1=OP.add)
            nc.vector.tensor_scalar(out=denom[:], in0=denom[:], scalar1=1.0,
                                    scalar2=None, op0=OP.max)
            rden = smallp.tile([1, 1], dt.float32)
            nc.vector.reciprocal(out=rden[:], in_=denom[:])
            scl = smallp.tile([1, 1], dt.float32)
            nc.vector.tensor_tensor(out=scl[:], in0=sglob[:], in1=rden[:],
                                    op=OP.mult)
            scale_rep = smallp.tile([P, 1], dt.float32)
            nc.gpsimd.partition_broadcast(scale_rep[:], scl[:])

            # ---------------- Phase B: stage x^T (bf16 hi/lo via DRAM) ------
            # scale is applied at the output stage, so staging only needs x.
            xh_dram = dramp.tile([BLOC, IF], dt.bfloat16, name="xh_dram")
            xl_dram = dramp.tile([BLOC, IF], dt.bfloat16, name="xl_dram")
            CCX = min(1024, IF)
            CC = min(2048, IF)
            for bt in range(n_bt):
                for cc in range(IF // CCX):
                    xs = xsp.tile([P, CCX], dt.float32, tag="xs")
                    nc.sync.dma_start(
                        out=xs, in_=x_in[bt * P:(bt + 1) * P,
                                         cc * CCX:(cc + 1) * CCX])
                    xhb = xsp.tile([P, CCX], dt.bfloat16, tag="xhb")
                    nc.scalar.copy(out=xhb[:], in_=xs[:])
                    xlb = xsp.tile([P, CCX], dt.bfloat16, tag="xlb")
                    nc.vector.tensor_tensor(out=xlb[:], in0=xs[:], in1=xhb[:],
                                            op=OP.subtract)
                    nc.sync.dma_start(
                        out=xh_dram[bt * P:(bt + 1) * P,
                                    cc * CCX:(cc + 1) * CCX],
                        in_=xhb[:])
                    nc.sync.dma_start(
                        out=xl_dram[bt * P:(bt + 1) * P,
                                    cc * CCX:(cc + 1) * CCX],
                        in_=xlb[:])
            xhT = bigp.tile([P, IF // P, BLOC], dt.bfloat16, tag="bigbuf")
            xlT = bigp.tile([P, IF // P, BLOC], dt.bfloat16, tag="bigbuf")
            for ic in range(IF // P):
                nc.sync.dma_start_transpose(
                    out=xhT[:, ic, :], in_=xh_dram[:, ic * P:(ic + 1) * P])
                nc.sync.dma_start_transpose(
                    out=xlT[:, ic, :], in_=xl_dram[:, ic * P:(ic + 1) * P])

            # ---------------- Phase C: quantize -> DRAM -> matmul ---------
            n_otg = OF // (4 * P)
            tern_drams = [dramp.tile([4 * P, IF], dt.bfloat16,
                                     name=f"tern_dram{g}")
                          for g in range(n_otg)]
            nt_rep = smallp.tile([P, 1], dt.float32)
            nc.vector.tensor_scalar(out=nt_rep[:], in0=t_rep[:], scalar1=-1.0,
                                    scalar2=None, op0=OP.mult)
            for wrow in range(OF // P):
                for cc in range(IF // CC):
                    wt = ternp.tile([P, CC], dt.float32, tag="wt", bufs=2)
                    nc.sync.dma_start(
                        out=wt, in_=w_in[wrow * P:(wrow + 1) * P,
                                         cc * CC:(cc + 1) * CC])
                    nb = ternp.tile([P, CC], dt.uint8, tag="nb", bufs=2)
                    nc.vector.tensor_scalar(out=nb[:], in0=wt[:],
                                            scalar1=nt_rep[:, :1],
                                            scalar2=None, op0=OP.is_lt)
                    tb = ternp.tile([P, CC], dt.bfloat16, tag="tb", bufs=2)
                    nc.vector.scalar_tensor_tensor(
                        out=tb[:], in0=wt[:], scalar=t_rep[:, :1], in1=nb[:],
                        op0=OP.is_gt, op1=OP.subtract)
                    nc.sync.dma_start(
                        out=tern_drams[wrow // 4][(wrow % 4) * P:
                                                  (wrow % 4 + 1) * P,
                                                  cc * CC:(cc + 1) * CC],
                        in_=tb[:])

            # bias for all o-tiles in one load: [128, n_ot]
            bias_all = smallp.tile([P, n_ot], dt.float32)
            nc.sync.dma_start(
                out=bias_all,
                in_=b_in.rearrange("(ot p) o -> p (ot o)", p=P))

            n_ic = IF // P
            OTG = 4
            for otg in range(n_ot // OTG):
                psb = [[pmm.tile([P, 512], dt.float32, tag="mm",
                                 space="PSUM", name=f"psb{otg}_{bh}_{g}")
                        for g in range(OTG)] for bh in range(n_bh)]
                for ic in range(n_ic):
                    ternT = ternp.tile([P, OTG * P], dt.bfloat16, tag="ternT",
                                       bufs=8)
                    nc.sync.dma_start_transpose(
                        out=ternT[:],
                        in_=tern_drams[otg][:, ic * P:(ic + 1) * P])
                    for bh in range(n_bh):
                        for g in range(OTG):
                            nc.tensor.matmul(
                                out=psb[bh][g][:],
                                lhsT=ternT[:, g * P:(g + 1) * P],
                                rhs=xhT[:, ic, bh * 512:(bh + 1) * 512],
                                start=(ic == 0), stop=False)
                            nc.tensor.matmul(
                                out=psb[bh][g][:],
                                lhsT=ternT[:, g * P:(g + 1) * P],
                                rhs=xlT[:, ic, bh * 512:(bh + 1) * 512],
                                start=False, stop=(ic == n_ic - 1))
                for g in range(OTG):
                    ot = otg * OTG + g
                    ysb = outp.tile([P, n_bh * 512], dt.float32, tag="ysb", bufs=2)
                    for bh in range(n_bh):
                        dst = ysb[:, bh * 512:(bh + 1) * 512]
                        if (g + bh) % 2 == 0:
                            nc.scalar.activation(dst, psb[bh][g][:],
                                                 AF.Identity,
                                                 bias=bias_all[:, ot:ot + 1],
                                                 scale=scale_rep[:, :1])
                        else:
                            nc.vector.tensor_scalar(
                                out=dst, in0=psb[bh][g][:],
                                scalar1=scale_rep[:, :1],
                                scalar2=bias_all[:, ot:ot + 1],
                                op0=OP.mult, op1=OP.add)
                    nc.sync.dma_start(
                        out=y_out[ot * P:(ot + 1) * P, :],
                        in_=ysb[:])
    nc.compile()
    return nc


_NC_CACHE = {}


def _get_nc():
    key = "full"
    if key not in _NC_CACHE:
        _NC_CACHE[key] = build()
    return _NC_CACHE[key]


def kernel(x, weight, bias):
    x = np.ascontiguousarray(np.asarray(x, dtype=np.float32))
    w = np.ascontiguousarray(np.asarray(weight, dtype=np.float32))
    b = np.ascontiguousarray(np.asarray(bias, dtype=np.float32))
    Bb, S, IF = x.shape
    OF = w.shape[0]
    xf = x.reshape(-1, IF)
    bloc = xf.shape[0] // NCORES
    rows = OF // NCORES
    nc = _get_nc()
    in_maps = []
    for c in range(NCORES):
        in_maps.append({
            "whist": np.ascontiguousarray(
                w[c * rows:(c + 1) * rows].reshape(P, -1)),
            "w": w,
            "x": np.ascontiguousarray(xf[c * bloc:(c + 1) * bloc]),
            "bias": b.reshape(-1, 1),
        })
    res = run_bass_kernel_spmd(nc, in_maps, core_ids=list(range(NCORES)))
    yT = np.concatenate([res.results[c]["y"] for c in range(NCORES)], axis=1)
    return np.ascontiguousarray(yT.T).reshape(Bb, S, OF)


# revision 24
# speedup vs baseline: 8836.6316x; 1.0279x over previous
"""BitLinear (ternary 1.58-bit quantized linear) Trainium2 kernel, 8 cores.

y = x @ (sign(w) * (|w| > t))^T * scale + bias
  t     = k-th smallest |w| (k = n/2, exact order statistic)
  scale = mean(|w| over kept weights)

Strategy (data-parallel over batch rows):
 - every core holds the full weight, a disjoint 1/8 row-shard of x, and a
   disjoint 1/8 of the weight for the threshold histogramming.
 - threshold: exact value-space bisection on fp32 |w| (10 rounds x 3
   midpoints, counts AllReduce'd across cores). All ops stay fp32/integer:
   DVE data converters are lossy (bf16 mantissa) and must be avoided.
 - matmul: x is pre-scaled by `scale`, split into bf16 hi+lo (exact-ternary
   weights in bf16), accumulated in fp32 PSUM -> fp32-class accuracy at
   bf16 PE rate. Stationary = transposed ternary tiles, moving = x^T.
 - psum [o,b] tiles are scaled+biased on the scalar engine, transposed back
   to [b,o] via PE transpose, and DMA'd straight to the output slab.
"""
import numpy as np
import concourse.bass as bass
import concourse.mybir as mybir
import concourse.tile as tile
from concourse import bacc
from concourse.bass_utils import run_bass_kernel_spmd
from concourse.masks import make_identity

dt = mybir.dt
OP = mybir.AluOpType
AX = mybir.AxisListType.X
AF = mybir.ActivationFunctionType

NCORES = 8
P = 128
SPARSITY = 0.5
BIG = 1e9


def _f32_bits_off(val, off):
    return float(np.uint32(int(np.float32(val).view(np.uint32)) + off).view(np.float32))


def build(IF=4096, OF=4096, BLOC=1024, ncores=NCORES, rounds=8, win=32768,
          no_collective=False, hist_cores=None):
    """Emit the SPMD program. Shapes: w [OF,IF], x-shard [BLOC,IF],
    whist [128, OF*IF/ncores/128], bias [OF,1] -> y [BLOC, OF]."""
    N = OF * IF
    K_RANK = int(N * SPARSITY)
    HF = N // (hist_cores or ncores) // P
    bound = 1.0 / np.sqrt(IF)
    wlo = _f32_bits_off(bound / 2, -win)
    whi = _f32_bits_off(bound / 2, +win)
    n_ot = OF // P          # output tiles
    n_ig = IF // 512        # weight column groups
    n_bt = BLOC // P        # x row tiles
    n_cc = IF // 1024       # x stage column chunks
    n_bh = BLOC // 512      # psum halves per o-tile
    assert BLOC % 512 == 0 and IF % 1024 == 0 and OF % P == 0

    nc = bacc.Bacc("TRN2", target_bir_lowering=False, debug=False,
                   num_devices=ncores)
    whist = nc.dram_tensor("whist", [P, HF], dt.float32, kind="ExternalInput").ap()
    w_in = nc.dram_tensor("w", [OF, IF], dt.float32, kind="ExternalInput").ap()
    x_in = nc.dram_tensor("x", [BLOC, IF], dt.float32, kind="ExternalInput").ap()
    b_in = nc.dram_tensor("bias", [OF, 1], dt.float32, kind="ExternalInput").ap()
    y_out = nc.dram_tensor("y", [OF, BLOC], dt.float32, kind="ExternalOutput").ap()

    with tile.TileContext(nc) as tc:
        with tc.tile_pool(name="bigp", bufs=2) as bigp, \
             tc.tile_pool(name="smallp", bufs=1) as smallp, \
             tc.tile_pool(name="xstage", bufs=2) as xsp, \
             tc.tile_pool(name="ternp", bufs=3) as ternp, \
             tc.tile_pool(name="outp", bufs=3) as outp, \
             tc.tile_pool(name="pmm", bufs=8, space="PSUM") as pmm, \
             tc.tile_pool(name="dramp", bufs=1, space="DRAM") as dramp:

            # ---------------- Phase A: threshold + scale ----------------
            a = bigp.tile([P, HF], dt.float32, tag="bigbuf")
            nc.sync.dma_start(out=a, in_=whist)
            # |w| in place (integer ALU, exact)
            nc.vector.tensor_scalar(out=a[:].bitcast(dt.int32),
                                    in0=a[:].bitcast(dt.int32),
                                    scalar1=0x7FFFFFFF, scalar2=None,
                                    op0=OP.bitwise_and)
            junk8 = smallp.tile([P, HF], dt.uint8, tag="junk")

            iota5 = smallp.tile([1, 5], dt.float32)
            for j in range(5):
                nc.vector.memset(iota5[:, j:j + 1], float(j))

            LH = smallp.tile([1, 2], dt.float32)
            nc.vector.memset(LH[:, 0:1], wlo)
            nc.vector.memset(LH[:, 1:2], whi)
            m_row = smallp.tile([1, 5], dt.float32)
            thr_rep = smallp.tile([P, 3], dt.float32)
            cnt128 = smallp.tile([P, 3], dt.float32)
            cntA = smallp.tile([P, 3], dt.float32)
            g_row = smallp.tile([1, 3], dt.float32)
            s_row = smallp.tile([1, 3], dt.float32)
            r11 = smallp.tile([1, 1], dt.float32)
            e_row = smallp.tile([1, 5], dt.float32)
            tmp5 = smallp.tile([1, 5], dt.float32)
            cle = smallp.tile([1, 1], dt.float32)
            gprev = smallp.tile([1, 1], dt.float32)
            zrow = smallp.tile([1, 3], dt.float32)

            bounce_in = dramp.tile([1, 3], dt.float32)
            bounce_out = dramp.tile([1, 3], dt.float32)
            rg = [list(range(ncores))]

            if rounds == 0:
                nc.vector.memset(cle[:], float(K_RANK))
            for rnd in range(rounds):
                nc.vector.tensor_copy(out=m_row[:, 0:1], in_=LH[:, 0:1])
                nc.vector.tensor_copy(out=m_row[:, 4:5], in_=LH[:, 1:2])
                nc.vector.tensor_tensor(out=m_row[:, 2:3], in0=LH[:, 0:1],
                                        in1=LH[:, 1:2], op=OP.add)
                nc.vector.tensor_scalar(out=m_row[:, 2:3], in0=m_row[:, 2:3],
                                        scalar1=0.5, scalar2=None, op0=OP.mult)
                nc.vector.tensor_tensor(out=m_row[:, 1:2], in0=m_row[:, 0:1],
                                        in1=m_row[:, 2:3], op=OP.add)
                nc.vector.tensor_scalar(out=m_row[:, 1:2], in0=m_row[:, 1:2],
                                        scalar1=0.5, scalar2=None, op0=OP.mult)
                nc.vector.tensor_tensor(out=m_row[:, 3:4], in0=m_row[:, 2:3],
                                        in1=m_row[:, 4:5], op=OP.add)
                nc.vector.tensor_scalar(out=m_row[:, 3:4], in0=m_row[:, 3:4],
                                        scalar1=0.5, scalar2=None, op0=OP.mult)
                nc.gpsimd.partition_broadcast(thr_rep[:], m_row[:, 1:4])
                for j in range(3):
                    nc.vector.tensor_scalar(
                        out=junk8[:], in0=a[:], scalar1=thr_rep[:, j:j + 1],
                        scalar2=0.0, op0=OP.is_le, op1=OP.add,
                        accum_out=cnt128[:, j:j + 1])
                import concourse.bass_isa as bass_isa
                nc.gpsimd.partition_all_reduce(cntA[:], cnt128[:], channels=P,
                                               reduce_op=bass_isa.ReduceOp.add)
                nc.sync.dma_start(out=bounce_in[:], in_=cntA[:1, :3])
                if no_collective:
                    nc.sync.dma_start(out=bounce_out[:], in_=bounce_in[:])
                else:
                    nc.gpsimd.collective_compute(
                        "AllReduce", OP.add, replica_groups=rg,
                        ins=[bounce_in[:]], outs=[bounce_out[:]])
                nc.sync.dma_start(out=g_row[:], in_=bounce_out[:])
                nc.vector.tensor_scalar(out=s_row[:], in0=g_row[:],
                                        scalar1=float(K_RANK), scalar2=None,
                                        op0=OP.is_lt)
                nc.vector.tensor_reduce(out=r11[:], in_=s_row[:], axis=AX,
                                        op=OP.add)
                nc.vector.tensor_scalar(out=e_row[:], in0=iota5[:],
                                        scalar1=r11[:, 0:1], scalar2=None,
                                        op0=OP.is_equal)
                nc.vector.tensor_tensor(out=tmp5[:], in0=m_row[:], in1=e_row[:],
                                        op=OP.mult)
                nc.vector.tensor_reduce(out=LH[:, 0:1], in_=tmp5[:], axis=AX,
                                        op=OP.add)
                nc.vector.tensor_scalar(out=e_row[:], in0=iota5[:],
                                        scalar1=r11[:, 0:1], scalar2=1.0,
                                        op0=OP.subtract, op1=OP.is_equal)
                nc.vector.tensor_tensor(out=tmp5[:], in0=m_row[:], in1=e_row[:],
                                        op=OP.mult)
                nc.vector.tensor_reduce(out=LH[:, 1:2], in_=tmp5[:], axis=AX,
                                        op=OP.add)
                nc.vector.tensor_scalar(out=zrow[:], in0=s_row[:], scalar1=BIG,
                                        scalar2=None, op0=OP.mult)
                nc.vector.tensor_tensor(out=zrow[:], in0=zrow[:], in1=g_row[:],
                                        op=OP.add)
                nc.vector.tensor_reduce(out=cle[:], in_=zrow[:], axis=AX,
                                        op=OP.min)
                if rnd == 0:
                    nc.vector.tensor_copy(out=gprev[:], in_=cle[:])
                else:
                    nc.vector.tensor_tensor(out=cle[:], in0=cle[:],
                                            in1=gprev[:], op=OP.min)
                    nc.vector.tensor_copy(out=gprev[:], in_=cle[:])

            t11 = smallp.tile([1, 1], dt.float32)
            nc.vector.tensor_copy(out=t11[:], in_=LH[:, 1:2])
            t_rep = smallp.tile([P, 1], dt.float32)
            nc.gpsimd.partition_broadcast(t_rep[:], t11[:])

            # S = sum(|w| where > t); in-place masked write (a is dead after)
            spart = smallp.tile([P, 1], dt.float32)
            nc.vector.scalar_tensor_tensor(
                out=a[:], in0=a[:], scalar=t_rep[:, :1], in1=a[:],
                op0=OP.is_gt, op1=OP.mult, accum_out=spart[:])
            spartA = smallp.tile([P, 1], dt.float32)
            import concourse.bass_isa as bass_isa
            nc.gpsimd.partition_all_reduce(spartA[:], spart[:], channels=P,
                                           reduce_op=bass_isa.ReduceOp.add)
            sloc = spartA
            sb_in = dramp.tile([1, 1], dt.float32)
            sb_out = dramp.tile([1, 1], dt.float32)
            nc.sync.dma_start(out=sb_in[:], in_=sloc[:1, :])
            if no_collective:
                nc.sync.dma_start(out=sb_out[:], in_=sb_in[:])
            else:
                nc.gpsimd.collective_compute(
                    "AllReduce", OP.add, replica_groups=rg,
                    ins=[sb_in[:]], outs=[sb_out[:]])
            sglob = smallp.tile([1, 1], dt.float32)
            nc.sync.dma_start(out=sglob[:], in_=sb_out[:])

            # scale = S / max(N - cnt_le, 1)
            denom = smallp.tile([1, 1], dt.float32)
            nc.vector.tensor_scalar(out=denom[:], in0=cle[:], scalar1=-1.0,
                                    scalar2=float(N), op0=OP.mult, op1=OP.add)
            nc.vector.tensor_scalar(out=denom[:], in0=denom[:], scalar1=1.0,
                                    scalar2=None, op0=OP.max)
            rden = smallp.tile([1, 1], dt.float32)
            nc.vector.reciprocal(out=rden[:], in_=denom[:])
            scl = smallp.tile([1, 1], dt.float32)
            nc.vector.tensor_tensor(out=scl[:], in0=sglob[:], in1=rden[:],
                                    op=OP.mult)
            scale_rep = smallp.tile([P, 1], dt.float32)
            nc.gpsimd.partition_broadcast(scale_rep[:], scl[:])

            # ---------------- Phase B: stage x^T (bf16 hi/lo via DRAM) ------
            # scale is applied at the output stage, so staging only needs x.
            xh_dram = dramp.tile([BLOC, IF], dt.bfloat16, name="xh_dram")
            xl_dram = dramp.tile([BLOC, IF], dt.bfloat16, name="xl_dram")
            CCX = min(1024, IF)
            CC = min(2048, IF)
            for bt in range(n_bt):
                for cc in range(IF // CCX):
                    xs = xsp.tile([P, CCX], dt.float32, tag="xs")
                    nc.sync.dma_start(
                        out=xs, in_=x_in[bt * P:(bt + 1) * P,
                                         cc * CCX:(cc + 1) * CCX])
                    xhb = xsp.tile([P, CCX], dt.bfloat16, tag="xhb")
                    nc.scalar.copy(out=xhb[:], in_=xs[:])
                    xlb = xsp.tile([P, CCX], dt.bfloat16, tag="xlb")
                    nc.vector.tensor_tensor(out=xlb[:], in0=xs[:], in1=xhb[:],
                                            op=OP.subtract)
                    nc.sync.dma_start(
                        out=xh_dram[bt * P:(bt + 1) * P,
                                    cc * CCX:(cc + 1) * CCX],
                        in_=xhb[:])
                    nc.sync.dma_start(
                        out=xl_dram[bt * P:(bt + 1) * P,
                                    cc * CCX:(cc + 1) * CCX],
                        in_=xlb[:])
            xhT = bigp.tile([P, IF // P, BLOC], dt.bfloat16, tag="bigbuf")
            xlT = bigp.tile([P, IF // P, BLOC], dt.bfloat16, tag="bigbuf")
            for ic in range(IF // P):
                nc.sync.dma_start_transpose(
                    out=xhT[:, ic, :], in_=xh_dram[:, ic * P:(ic + 1) * P])
                nc.sync.dma_start_transpose(
                    out=xlT[:, ic, :], in_=xl_dram[:, ic * P:(ic + 1) * P])

            # ---------------- Phase C: quantize -> DRAM -> matmul ---------
            n_otg = OF // (4 * P)
            tern_drams = [dramp.tile([4 * P, IF], dt.bfloat16,
                                     name=f"tern_dram{g}")
                          for g in range(n_otg)]
            nt_rep = smallp.tile([P, 1], dt.float32)
            nc.vector.tensor_scalar(out=nt_rep[:], in0=t_rep[:], scalar1=-1.0,
                                    scalar2=None, op0=OP.mult)
            for wrow in range(OF // P):
                for cc in range(IF // CC):
                    wt = ternp.tile([P, CC], dt.float32, tag="wt", bufs=2)
                    nc.sync.dma_start(
                        out=wt, in_=w_in[wrow * P:(wrow + 1) * P,
                                         cc * CC:(cc + 1) * CC])
                    nb = ternp.tile([P, CC], dt.uint8, tag="nb", bufs=2)
                    nc.vector.tensor_scalar(out=nb[:], in0=wt[:],
                                            scalar1=nt_rep[:, :1],
                                            scalar2=None, op0=OP.is_lt)
                    tb = ternp.tile([P, CC], dt.bfloat16, tag="tb", bufs=2)
                    nc.vector.scalar_tensor_tensor(
                        out=tb[:], in0=wt[:], scalar=t_rep[:, :1], in1=nb[:],
                        op0=OP.is_gt, op1=OP.subtract)
                    nc.sync.dma_start(
                        out=tern_drams[wrow // 4][(wrow % 4) * P:
                                                  (wrow % 4 + 1) * P,
                                                  cc * CC:(cc + 1) * CC],
                        in_=tb[:])

            # bias for all o-tiles in one load: [128, n_ot]
            bias_all = smallp.tile([P, n_ot], dt.float32)
            nc.sync.dma_start(
                out=bias_all,
                in_=b_in.rearrange("(ot p) o -> p (ot o)", p=P))

            n_ic = IF // P
            OTG = 4
            for otg in range(n_ot // OTG):
                psb = [[pmm.tile([P, 512], dt.float32, tag="mm",
                                 space="PSUM", name=f"psb{otg}_{bh}_{g}")
                        for g in range(OTG)] for bh in range(n_bh)]
                for ic in range(n_ic):
                    ternT = ternp.tile([P, OTG * P], dt.bfloat16, tag="ternT",
                                       bufs=8)
                    nc.sync.dma_start_transpose(
                        out=ternT[:],
                        in_=tern_drams[otg][:, ic * P:(ic + 1) * P])
                    for bh in range(n_bh):
                        for g in range(OTG):
                            nc.tensor.matmul(
                                out=psb[bh][g][:],
                                lhsT=ternT[:, g * P:(g + 1) * P],
                                rhs=xhT[:, ic, bh * 512:(bh + 1) * 512],
                                start=(ic == 0), stop=False)
                            nc.tensor.matmul(
                                out=psb[bh][g][:],
                                lhsT=ternT[:, g * P:(g + 1) * P],
                                rhs=xlT[:, ic, bh * 512:(bh + 1) * 512],
                                start=False, stop=(ic == n_ic - 1))
                for g in range(OTG):
                    ot = otg * OTG + g
                    ysb = outp.tile([P, n_bh * 512], dt.float32, tag="ysb", bufs=2)
                    for bh in range(n_bh):
                        dst = ysb[:, bh * 512:(bh + 1) * 512]
                        if (g + bh) % 2 == 0:
                            nc.scalar.activation(dst, psb[bh][g][:],
                                                 AF.Identity,
                                                 bias=bias_all[:, ot:ot + 1],
                                                 scale=scale_rep[:, :1])
                        else:
                            nc.vector.tensor_scalar(
                                out=dst, in0=psb[bh][g][:],
                                scalar1=scale_rep[:, :1],
                                scalar2=bias_all[:, ot:ot + 1],
                                op0=OP.mult, op1=OP.add)
                    nc.sync.dma_start(
                        out=y_out[ot * P:(ot + 1) * P, :],
                        in_=ysb[:])
    nc.compile()
    return nc


_NC_CACHE = {}


def _get_nc():
    key = "full"
    if key not in _NC_CACHE:
        _NC_CACHE[key] = build()
    return _NC_CACHE[key]


def kernel(x, weight, bias):
    x = np.ascontiguousarray(np.asarray(x, dtype=np.float32))
    w = np.ascontiguousarray(np.asarray(weight, dtype=np.float32))
    b = np.ascontiguousarray(np.asarray(bias, dtype=np.float32))
    Bb, S, IF = x.shape
    OF = w.shape[0]
    xf = x.reshape(-1, IF)
    bloc = xf.shape[0] // NCORES
    rows = OF // NCORES
    nc = _get_nc()
    in_maps = []
    for c in range(NCORES):
        in_maps.append({
            "whist": np.ascontiguousarray(
                w[c * rows:(c + 1) * rows].reshape(P, -1)),
            "w": w,
            "x": np.ascontiguousarray(xf[c * bloc:(c + 1) * bloc]),
            "bias": b.reshape(-1, 1),
        })
    res = run_bass_kernel_spmd(nc, in_maps, core_ids=list(range(NCORES)))
    yT = np.concatenate([res.results[c]["y"] for c in range(NCORES)], axis=1)
    return np.ascontiguousarray(yT.T).reshape(Bb, S, OF)
